# revision 16
# baseline (speedup 1.0000x reference)
"""Trainium2 Bass kernel for SoftAlignAttentionMixed.

Shapes: x, cond (4, 512, 2048); 8 projection weights (512, 512) + biases.
Computation = local windowed attention (W=16) + global RoPE attention,
mixed per-batch by sqrt(1-t/999) / sqrt(t/999).

Sharding: 8 cores = (batch b, T-half) grid. Each core computes, for its
batch, the global attention for 1024 contiguous query positions and the
local attention for the 1024 strided query positions {t : t%32 in
[16*half, 16*half+16)}. The strided split matches the reference's
torch-faithful channel scramble of the local branch: its
(B,H,T,D).view(B,C,T) maps head h, query t=32a+r, dim d to channel
64h+a, position 64r+d — so output columns [hs, hs+1024) need exactly the
local-attention results for queries with t%32 in the core's residue set.
No collectives; host gathers (C, 1024) output slabs.

On-device layout notes:
 - Projections keep channels-first (C, T): matmul lhsT = W^T tile, rhs =
   input, inputs/weights in bf16, accumulate f32, outputs stored f32r.
 - Biases applied by the PE via a bias row appended to W^T times a ones row.
 - Local K/V span padded columns [t=-64, t=2112); edge columns are zeroed on
   device, reproducing the reference's zero-padding softmax semantics
   (pad logits 0, pad v rows 0, denominators include pads).
 - V is projected directly into transposed (T, C) layout (lhsT = cond tile,
   rhs = W^T), with an extra ones-column per head so the attention AV
   matmul also produces softmax denominators (row 64 of its PSUM out).
 - Scores are computed transposed: S_T[tk, tq] = k_tile^T @ q (f32r), exp on
   ACT, AV contracts over tk partitions; normalization = reciprocal of the
   denominator row, rank-1 PE broadcast, one DVE multiply.
 - RoPE: interleaved pairs are adjacent partitions; pair swap via DVE
   stream_shuffle; out = x*C2 + swap(x)*S2 with host tables.
 - The local-branch scramble is materialized per head with 16 PE
   transposes of 64x64 column-strided blocks of the av map.
 - t-mixing folded into host-scaled output weights; both output projections
   accumulate into one PSUM tile.
"""

import numpy as np

B, C, T, H, W, TS = 4, 512, 2048, 8, 16, 1000
D = C // H            # 64
TH = T // 2           # 1024 queries per core (per branch)
KL = T + 128          # 2176 local key cols (t = col-64, 64-col pads)
SCALE = D ** -0.5
N_CORES = 8
NEG = -1.0e30

_prog_cache = {}


def _build_program():
    if "nc" in _prog_cache:
        return _prog_cache["nc"]

    import concourse.bacc as bacc
    import concourse.mybir as mybir
    import concourse.tile as tile

    f32 = mybir.dt.float32
    f32r = mybir.dt.float32r
    bf16 = mybir.dt.bfloat16
    EXP = mybir.ActivationFunctionType.Exp
    IDENT = mybir.ActivationFunctionType.Identity

    nc = bacc.Bacc("TRN2", target_bir_lowering=False, debug=False,
                   num_devices=N_CORES)

    # ---- DRAM I/O (per-core tensors; same program on all cores) ----
    d_x = nc.dram_tensor("x", [C, TH], bf16, kind="ExternalInput")
    d_xq = nc.dram_tensor("xq", [C, TH], bf16, kind="ExternalInput")
    d_cond = nc.dram_tensor("cond", [C, T], bf16, kind="ExternalInput")
    d_w = {}
    for name in ("wql", "wkl", "wvl", "wqg", "wkg", "wvg"):
        d_w[name] = nc.dram_tensor(name, [C + 1, C], bf16,
                                   kind="ExternalInput")
    d_wog = nc.dram_tensor("wog", [C, C], f32r, kind="ExternalInput")
    d_wol = nc.dram_tensor("wol", [C, C], bf16, kind="ExternalInput")
    d_bo = nc.dram_tensor("bo", [C, 1], f32, kind="ExternalInput")
    d_cosq = nc.dram_tensor("cosq", [128, TH], bf16, kind="ExternalInput")
    d_sinq = nc.dram_tensor("sinq", [128, TH], bf16, kind="ExternalInput")
    d_cosk = nc.dram_tensor("cosk", [128, T], bf16, kind="ExternalInput")
    d_sink = nc.dram_tensor("sink", [128, T], bf16, kind="ExternalInput")
    d_mask = nc.dram_tensor("mask", [128, 1280], bf16, kind="ExternalInput")
    d_ident = nc.dram_tensor("ident", [128, 64], f32r, kind="ExternalInput")
    d_out = nc.dram_tensor("out", [C, TH], f32, kind="ExternalOutput")

    SWAP_ADJ = [i ^ 1 for i in range(32)]
    NT_G = T // 128        # 16 global key tiles
    NT_L = KL // 128       # 17 local key tiles (last has 64 valid rows)

    with tile.TileContext(nc) as tc:
        with tc.tile_pool(name="persist", bufs=1) as pp, \
             tc.tile_pool(name="wpool", bufs=1) as wp, \
             tc.tile_pool(name="chunk", bufs=1) as cp, \
             tc.tile_pool(name="psA", bufs=2, space="PSUM") as psA, \
             tc.tile_pool(name="psB", bufs=2, space="PSUM") as psB, \
             tc.tile_pool(name="psC", bufs=2, space="PSUM") as psC:

            # ---------- persistent inputs ----------
            def load_rows(dram, rows, cols, dt):
                tiles = []
                nt = (rows + 127) // 128
                for i in range(nt):
                    p = min(128, rows - i * 128)
                    t_ = pp.tile([p, cols], dt, tag=f"ld_{dram.name}_{i}",
                                 name=f"ld_{dram.name}_{i}")
                    nc.sync.dma_start(t_[:], dram.ap()[i * 128:i * 128 + p, :])
                    tiles.append(t_)
                return tiles

            x_sb = load_rows(d_x, C, TH, bf16)          # 4x[128,TH]
            xq_sb = load_rows(d_xq, C, TH, bf16)
            cond_sb = load_rows(d_cond, C, T, bf16)     # 4x[128,T]
            cosq = load_rows(d_cosq, 128, TH, bf16)[0]
            sinq = load_rows(d_sinq, 128, TH, bf16)[0]
            cosk = load_rows(d_cosk, 128, T, bf16)[0]
            sink = load_rows(d_sink, 128, T, bf16)[0]
            mask_sb = load_rows(d_mask, 128, 1280, bf16)[0]
            bo_sb = load_rows(d_bo, C, 1, f32)          # 4x[128,1]
            ident = load_rows(d_ident, 128, 64, f32r)[0]

            ones_f32 = pp.tile([1, 64], f32, name="ones_f32")
            nc.vector.memset(ones_f32[:], 1.0)
            ones64 = pp.tile([1, 64], f32r, name="ones64")
            nc.vector.tensor_copy(ones64[:], ones_f32[:])
            ones512 = pp.tile([1, 512], bf16, name="ones512")
            nc.vector.memset(ones512[:], 1.0)
            onescol = pp.tile([128, 4], f32, name="onescol")
            nc.vector.memset(onescol[:], 1.0)
            zerot = pp.tile([128, 256], f32, name="zerot")
            nc.vector.memset(zerot[:], 0.0)

            # attention outputs, assembled in (C, TH) layout
            av_g = [pp.tile([128, TH], f32r, tag=f"avg{i}", name=f"avg{i}")
                    for i in range(4)]

            for hg in range(2):           # head group: 4 heads, 256 channels
                co = hg * 256

                # ---------- this head-group's weight slices ----------
                w_sb = {}
                for name in ("wql", "wkl", "wvl", "wqg", "wkg", "wvg"):
                    tiles = []
                    for i in range(4):
                        t_ = wp.tile([128, 256], bf16, tag=f"w{name}_{i}",
                                     name=f"w{name}_{i}")
                        nc.sync.dma_start(
                            t_[:],
                            d_w[name].ap()[i * 128:(i + 1) * 128, co:co + 256])
                        tiles.append(t_)
                    t_ = wp.tile([1, 256], bf16, tag=f"w{name}_b",
                                 name=f"w{name}_b")
                    nc.sync.dma_start(t_[:], d_w[name].ap()[C:C + 1,
                                                            co:co + 256])
                    tiles.append(t_)
                    w_sb[name] = tiles

                # ---------- (C,T)-layout projections ----------
                def proj_ct(wname, src_sb, tlen, dst_tiles, dst_off=0,
                            scale=None, rope=None):
                    w = w_sb[wname]
                    for o in range(2):
                        for t0 in range(0, tlen, 512):
                            n = min(512, tlen - t0)
                            ps = psA.tile([128, n], f32, tag="proj",
                                          name="proj_ps")
                            for ct in range(4):
                                nc.tensor.matmul(
                                    ps[:], w[ct][:, o * 128:(o + 1) * 128],
                                    src_sb[ct][:, t0:t0 + n],
                                    start=(ct == 0), stop=False)
                            nc.tensor.matmul(
                                ps[:], w[4][:, o * 128:(o + 1) * 128],
                                ones512[:, 0:n], start=False, stop=True)
                            dst = dst_tiles[o][:, dst_off + t0:
                                               dst_off + t0 + n]
                            if rope is None:
                                if scale is not None:
                                    nc.scalar.mul(dst, ps[:], scale)
                                else:
                                    nc.vector.tensor_copy(dst, ps[:])
                            else:
                                ctab, stab = rope
                                if scale is not None:
                                    nc.scalar.mul(dst, ps[:], scale)
                                else:
                                    nc.scalar.copy(dst, ps[:])
                                shuf = cp.tile([128, n], f32r, tag="rshuf",
                                               name="rshuf", bufs=1)
                                nc.vector.stream_shuffle(
                                    shuf[:].bitcast(f32), dst.bitcast(f32),
                                    SWAP_ADJ)
                                nc.vector.tensor_mul(dst, dst,
                                                     ctab[:, t0:t0 + n])
                                nc.vector.tensor_mul(shuf[:], shuf[:],
                                                     stab[:, t0:t0 + n])
                                nc.vector.tensor_add(dst, dst, shuf[:])

                q_l = [cp.tile([128, TH], f32r, tag=f"ql{o}", name=f"ql{o}")
                       for o in range(2)]
                av_l = [cp.tile([128, TH], f32r, tag=f"avl{i}",
                                name=f"avl{i}_{hg}") for i in range(2)]
                k_l = [cp.tile([128, KL], f32r, tag=f"kl{o}", name=f"kl{o}")
                       for o in range(2)]
                q_g = [cp.tile([128, TH], f32r, tag=f"qg{o}", name=f"qg{o}")
                       for o in range(2)]
                k_g = [cp.tile([128, T], f32r, tag=f"kg{o}", name=f"kg{o}")
                       for o in range(2)]

                # local K occupies columns [64, 2112) = t + 64; zero edges
                for o in range(2):
                    nc.vector.tensor_copy(k_l[o][:, 0:64], zerot[:, 0:64])
                    nc.vector.tensor_copy(k_l[o][:, T + 64:KL],
                                          zerot[:, 0:64])

                proj_ct("wql", xq_sb, TH, q_l, scale=SCALE)
                proj_ct("wkl", cond_sb, T, k_l, dst_off=64)
                proj_ct("wqg", x_sb, TH, q_g, scale=SCALE, rope=(cosq, sinq))
                proj_ct("wkg", cond_sb, T, k_g, rope=(cosk, sink))

                # ---------- transposed V projections (T, C+ones) ----------
                def proj_vt(wname, tlen, col_off):
                    # v tile tt rows p cover key-cols [128tt, 128tt+p);
                    # cond col = key-col - col_off.
                    w = w_sb[wname]
                    nt = (tlen + 127) // 128
                    tiles = []
                    for tt in range(nt):
                        p = min(128, tlen - tt * 128)
                        vt = cp.tile([128, 4 * 65], f32r,
                                     tag=f"vt_{wname}_{tt}",
                                     name=f"vt_{wname}_{tt}")
                        cstart = max(0, tt * 128 - col_off)
                        cend = min(T, tt * 128 + p - col_off)
                        ro = cstart - (tt * 128 - col_off)
                        m = cend - cstart
                        ps = psA.tile([128, 256], f32, tag="proj",
                                      name="proj_ps2")
                        for ct in range(4):
                            nc.tensor.matmul(
                                ps[ro:ro + m, :],
                                cond_sb[ct][:, cstart:cend],
                                w[ct][:, :], start=(ct == 0), stop=False)
                        nc.tensor.matmul(
                            ps[ro:ro + m, :], ones512[:, 0:m], w[4][:, :],
                            start=False, stop=True)
                        vda = vt[:, :].rearrange("p (h u) -> p h u", u=65)
                        nc.vector.tensor_copy(
                            vda[ro:ro + m, :, 0:64],
                            ps[ro:ro + m, :].rearrange("p (h d) -> p h d",
                                                       d=64))
                        # zero pad rows of the v columns
                        zview = zerot[:, 0:256].rearrange(
                            "p (h d) -> p h d", d=64)
                        if ro > 0:
                            nc.vector.tensor_copy(vda[0:ro, :, 0:64],
                                                  zview[0:ro])
                        if ro + m < p:
                            nc.vector.tensor_copy(vda[ro + m:p, :, 0:64],
                                                  zview[ro + m:p])
                        nc.vector.tensor_copy(
                            vda[0:p, :, 64:65],
                            onescol[0:p, :].rearrange("p (h u) -> p h u", u=1))
                        tiles.append(vt)
                    return tiles

                v_lT = proj_vt("wvl", KL, 64)
                v_gT = proj_vt("wvg", T, 0)

                # ---------- local attention (4 heads, W=16 band) ----------
                # query j (0..1023) = strided position t = 32*(j//16) + r0
                # + j%16; key tiles per 256-query block: 5 tiles of 128.
                for hh in range(4):
                    r0 = (hh % 2) * 64
                    qt = hh // 2
                    for qb in range(4):
                        j0 = qb * 256
                        pe = cp.tile([128, 1280], f32r, tag="lpe", name="lpe",
                                     bufs=1)
                        for ti in range(5):
                            ks = qb * 512 + ti * 128
                            kw = min(128, KL - ks)
                            ps_s = psB.tile([128, 256], f32, tag="gs",
                                            name="ls_ps")
                            nc.tensor.matmul(
                                ps_s[0:kw, :],
                                k_l[qt][r0:r0 + 64, ks:ks + kw],
                                q_l[qt][r0:r0 + 64, j0:j0 + 256],
                                start=True, stop=True)
                            nc.vector.tensor_add(
                                pe[:, ti * 256:(ti + 1) * 256], ps_s[:],
                                mask_sb[:, ti * 256:(ti + 1) * 256])
                        nc.scalar.activation(pe[:], pe[:], EXP)
                        ps_av = psC.tile([65, 512], f32, tag="av",
                                         name="lav_ps")
                        for ti in range(5):
                            ks = qb * 512 + ti * 128
                            kw = min(128, KL - ks)
                            nc.tensor.matmul(
                                ps_av[:, 0:256],
                                v_lT[ks // 128][0:kw, hh * 65:(hh + 1) * 65],
                                pe[0:kw, ti * 256:(ti + 1) * 256],
                                start=(ti == 0), stop=(ti == 4))
                        rl = cp.tile([1, 512], f32r, tag="rec", name="lrec",
                                     bufs=1)
                        with nc.allow_low_precision(reason="f32r recip"):
                            nc.vector.reciprocal(rl[0:1, 0:256],
                                                 ps_av[64:65, 0:256])
                        ps_bc = psC.tile([64, 512], f32, tag="bc",
                                         name="lbc_ps", bufs=1)
                        nc.tensor.matmul(ps_bc[:, 0:256], ones64[:],
                                         rl[0:1, 0:256],
                                         start=True, stop=True)
                        bc = cp.tile([64, 512], f32, tag="bcs", name="lbcs",
                                     bufs=1)
                        nc.scalar.copy(bc[:, 0:256], ps_bc[:, 0:256])
                        nc.vector.tensor_mul(
                            av_l[hh // 2][r0:r0 + 64, j0:j0 + 256],
                            ps_av[0:64, 0:256], bc[:, 0:256])

                # ---------- global attention (4 heads, full T keys) --------
                for hh in range(4):
                    r0 = (hh % 2) * 64
                    qt = hh // 2
                    for qb in range(2):
                        j0 = qb * 512
                        ps_av = psC.tile([65, 512], f32, tag="av",
                                         name="gav_ps")
                        for kt in range(NT_G):
                            ps_s = psB.tile([128, 512], f32, tag="gs",
                                            name="gs_ps")
                            nc.tensor.matmul(
                                ps_s[:],
                                k_g[qt][r0:r0 + 64, kt * 128:(kt + 1) * 128],
                                q_g[qt][r0:r0 + 64, j0:j0 + 512],
                                start=True, stop=True)
                            pe = cp.tile([128, 512], f32r, tag="gpe",
                                         name="gpe", bufs=2)
                            nc.scalar.activation(pe[:], ps_s[:], EXP)
                            nc.tensor.matmul(
                                ps_av[:],
                                v_gT[kt][:, hh * 65:(hh + 1) * 65],
                                pe[:],
                                start=(kt == 0), stop=(kt == NT_G - 1))
                        rl = cp.tile([1, 512], f32r, tag="rec", name="grec",
                                     bufs=1)
                        with nc.allow_low_precision(reason="f32r recip"):
                            nc.vector.reciprocal(rl[:], ps_av[64:65, :])
                        ps_bc = psC.tile([64, 512], f32, tag="bc",
                                         name="gbc_ps", bufs=1)
                        nc.tensor.matmul(ps_bc[:], ones64[:], rl[:],
                                         start=True, stop=True)
                        bc = cp.tile([64, 512], f32, tag="bcs", name="gbcs",
                                     bufs=1)
                        nc.scalar.copy(bc[:], ps_bc[:])
                        ci = (co + hh * 64) // 128
                        nc.vector.tensor_mul(
                            av_g[ci][r0:r0 + 64, j0:j0 + 512],
                            ps_av[0:64, :], bc[:])

                # ---------- local-branch channel scramble ----------
                # scr[64h + a, 64rr + d] = av_l[64h + d, 16a + rr]
                scr = [cp.tile([128, TH], bf16, tag=f"scr{o}",
                               name=f"scr{o}_{hg}", bufs=2)
                       for o in range(2)]
                for hh in range(4):
                    r0 = (hh % 2) * 64
                    src = av_l[hh // 2][r0:r0 + 64, :].rearrange(
                        "p (a rr) -> p rr a", rr=16)
                    for rr8 in range(2):
                        ps_t = psB.tile([64, 512], f32r, tag="gs",
                                        name="scr_ps")
                        for k in range(8):
                            rr = rr8 * 8 + k
                            nc.tensor.transpose(
                                ps_t[:, k * 64:(k + 1) * 64],
                                src[:, rr, :], ident[r0:r0 + 64, :])
                        nc.vector.tensor_copy(
                            scr[hh // 2][r0:r0 + 64,
                                         rr8 * 512:(rr8 + 1) * 512],
                            ps_t[:])

                # scr feeds the local output projection in place of av_l
                if hg == 0:
                    scr_all = [scr[0], scr[1], None, None]
                else:
                    scr_all = scr_all[:2] + [scr[0], scr[1]]

            # ---------- output projections (accumulate global + local) ----
            wog_sb = []
            wol_sb = []
            for i in range(4):
                t_ = wp.tile([128, 512], f32r, tag=f"wog{i}", name=f"wog{i}")
                nc.sync.dma_start(t_[:], d_wog.ap()[i * 128:(i + 1) * 128, :])
                wog_sb.append(t_)
                t_ = wp.tile([128, 512], bf16, tag=f"wol{i}", name=f"wol{i}")
                nc.sync.dma_start(t_[:], d_wol.ap()[i * 128:(i + 1) * 128, :])
                wol_sb.append(t_)

            for o in range(4):
                for tb in range(2):
                    t0 = tb * 512
                    ps = psA.tile([128, 512], f32, tag="proj", name="fin_ps")
                    for ct in range(4):
                        nc.tensor.matmul(
                            ps[:], wog_sb[ct][:, o * 128:(o + 1) * 128],
                            av_g[ct][:, t0:t0 + 512],
                            start=(ct == 0), stop=False)
                    for ct in range(4):
                        nc.tensor.matmul(
                            ps[:], wol_sb[ct][:, o * 128:(o + 1) * 128],
                            scr_all[ct][:, t0:t0 + 512],
                            start=False, stop=(ct == 3))
                    ot = cp.tile([128, 512], f32, tag="rshuf", name="outt",
                                 bufs=1)
                    nc.scalar.activation(ot[:], ps[:], IDENT,
                                         bias=bo_sb[o][:])
                    nc.sync.dma_start(d_out.ap()[o * 128:(o + 1) * 128,
                                                 t0:t0 + 512], ot[:])

    nc.compile()
    _prog_cache["nc"] = nc
    return nc


def _rope_tables():
    inv = 1.0 / (10000.0 ** (np.arange(0, D, 2, dtype=np.float64) / D))
    pos = np.arange(T, dtype=np.float64)
    ang = (pos[None, :] * inv[:, None]).astype(np.float32)  # (32, T)
    cosb = np.cos(ang).astype(np.float32)
    sinb = np.sin(ang).astype(np.float32)
    rows = (np.arange(128) % D) // 2
    c2 = cosb[rows]                       # (128, T)
    sign = np.where(np.arange(128) % 2 == 0, -1.0, 1.0).astype(np.float32)
    s2 = sinb[rows] * sign[:, None]
    return c2, s2


def _band_mask(r0):
    # S_T chunk ti (key cols 512qb+128ti + i) x query col j0+j:
    # t_q = 512qb + 32*(j//16) + r0 + j%16; key t = 512qb + 128ti + i - 64.
    # In-window iff key t - t_q in [-8, 7].
    m = np.full((128, 1280), NEG, dtype=np.float32)
    i = np.arange(128)[:, None]
    j = np.arange(256)[None, :]
    tq = 32 * (j // 16) + r0 + (j % 16)
    for ti in range(5):
        diff = (128 * ti + i - 64) - tq
        m[:, ti * 256:(ti + 1) * 256] = np.where(
            (diff >= -8) & (diff <= 7), 0.0, NEG)
    return m


def kernel(**inputs):
    import ml_dtypes
    bf = ml_dtypes.bfloat16

    x = np.ascontiguousarray(inputs["x"], dtype=np.float32)
    cond = np.ascontiguousarray(inputs["cond"], dtype=np.float32)
    t = np.asarray(inputs["t"]).astype(np.float64)

    def wT(w, b):
        return np.concatenate(
            [np.asarray(w, np.float32).T,
             np.asarray(b, np.float32)[None, :]], axis=0).astype(bf)

    w_ql = wT(inputs["lq_w"], inputs["lq_b"])
    w_kl = wT(inputs["lk_w"], inputs["lk_b"])
    w_vl = wT(inputs["lv_w"], inputs["lv_b"])
    w_qg = wT(inputs["gq_w"], inputs["gq_b"])
    w_kg = wT(inputs["gk_w"], inputs["gk_b"])
    w_vg = wT(inputs["gv_w"], inputs["gv_b"])

    t_norm = t / np.float64(TS - 1)
    sg = np.sqrt(1.0 - t_norm).astype(np.float32)   # (B,)
    sl = np.sqrt(t_norm).astype(np.float32)

    c2, s2 = _rope_tables()
    masks = [_band_mask(0).astype(bf), _band_mask(16).astype(bf)]
    ident = np.vstack([np.eye(64), np.eye(64)]).astype(np.float32)

    go_w = np.asarray(inputs["go_w"], np.float32)
    lo_w = np.asarray(inputs["lo_w"], np.float32)
    go_b = np.asarray(inputs["go_b"], np.float32)
    lo_b = np.asarray(inputs["lo_b"], np.float32)

    x_bf = x.astype(bf)
    cond_bf = cond.astype(bf)

    # strided local query columns: half h takes t with t%32 in [16h, 16h+16)
    tcols = np.arange(T).reshape(T // 32, 32)
    qsel = [tcols[:, 0:16].ravel(), tcols[:, 16:32].ravel()]

    in_maps = []
    for core in range(N_CORES):
        b = core // 2
        half = core % 2
        hs = half * TH
        in_maps.append({
            "x": np.ascontiguousarray(x_bf[b][:, hs:hs + TH]),
            "xq": np.ascontiguousarray(x_bf[b][:, qsel[half]]),
            "cond": cond_bf[b],
            "wql": w_ql, "wkl": w_kl, "wvl": w_vl,
            "wqg": w_qg, "wkg": w_kg, "wvg": w_vg,
            "wog": (go_w.T * sg[b]).copy(),
            "wol": (lo_w.T * sl[b]).astype(bf),
            "bo": (sg[b] * go_b + sl[b] * lo_b).reshape(C, 1).copy(),
            "cosq": np.ascontiguousarray(c2[:, hs:hs + TH]).astype(bf),
            "sinq": np.ascontiguousarray(s2[:, hs:hs + TH]).astype(bf),
            "cosk": c2.astype(bf),
            "sink": s2.astype(bf),
            "mask": masks[half],
            "ident": ident,
        })

    nc = _build_program()
    from concourse.bass_utils import run_bass_kernel_spmd
    res = run_bass_kernel_spmd(nc, in_maps, list(range(N_CORES)))

    out = np.empty((B, C, T), np.float32)
    for core in range(N_CORES):
        b = core // 2
        hs = (core % 2) * TH
        out[b][:, hs:hs + TH] = res.results[core]["out"]
    return out


# revision 24
# speedup vs baseline: 1.0324x; 1.0324x over previous
"""Trainium2 Bass kernel for SoftAlignAttentionMixed.

Shapes: x, cond (4, 512, 2048); 8 projection weights (512, 512) + biases.
Computation = local windowed attention (W=16) + global RoPE attention,
mixed per-batch by sqrt(1-t/999) / sqrt(t/999).

Sharding: 8 cores = (batch b, T-half) grid. Each core computes, for its
batch, the global attention for 1024 contiguous query positions and the
local attention for the 1024 strided query positions {t : t%32 in
[16*half, 16*half+16)}. The strided split matches the reference's
torch-faithful channel scramble of the local branch: its
(B,H,T,D).view(B,C,T) maps head h, query t=32a+r, dim d to channel
64h+a, position 64r+d — so output columns [hs, hs+1024) need exactly the
local-attention results for queries with t%32 in the core's residue set.
No collectives; host gathers (C, 1024) output slabs.

On-device layout notes:
 - Projections keep channels-first (C, T): matmul lhsT = W^T tile, rhs =
   input, inputs/weights in bf16, accumulate f32, outputs stored f32r.
 - Biases applied by the PE via a bias row appended to W^T times a ones row.
 - Local K/V span padded columns [t=-64, t=2112); edge columns are zeroed on
   device, reproducing the reference's zero-padding softmax semantics
   (pad logits 0, pad v rows 0, denominators include pads).
 - V is projected directly into transposed (T, C) layout (lhsT = cond tile,
   rhs = W^T), with an extra ones-column per head so the attention AV
   matmul also produces softmax denominators (row 64 of its PSUM out).
 - Scores are computed transposed: S_T[tk, tq] = k_tile^T @ q (f32r), exp on
   ACT, AV contracts over tk partitions; normalization = reciprocal of the
   denominator row, rank-1 PE broadcast, one DVE multiply.
 - RoPE: interleaved pairs are adjacent partitions; pair swap via DVE
   stream_shuffle; out = x*C2 + swap(x)*S2 with host tables.
 - The local-branch scramble is materialized per head with 16 PE
   transposes of 64x64 column-strided blocks of the av map.
 - t-mixing folded into host-scaled output weights; both output projections
   accumulate into one PSUM tile.
"""

import numpy as np

B, C, T, H, W, TS = 4, 512, 2048, 8, 16, 1000
D = C // H            # 64
TH = T // 2           # 1024 queries per core (per branch)
KL = T + 128          # 2176 local key cols (t = col-64, 64-col pads)
SCALE = D ** -0.5
N_CORES = 8
NEG = -1.0e30

_prog_cache = {}


def _build_program():
    if "nc" in _prog_cache:
        return _prog_cache["nc"]

    import concourse.bacc as bacc
    import concourse.mybir as mybir
    import concourse.tile as tile

    f32 = mybir.dt.float32
    f32r = mybir.dt.float32r
    bf16 = mybir.dt.bfloat16
    EXP = mybir.ActivationFunctionType.Exp
    IDENT = mybir.ActivationFunctionType.Identity

    nc = bacc.Bacc("TRN2", target_bir_lowering=False, debug=False,
                   num_devices=N_CORES)

    # ---- DRAM I/O (per-core tensors; same program on all cores) ----
    d_x = nc.dram_tensor("x", [C, TH], bf16, kind="ExternalInput")
    d_xq = nc.dram_tensor("xq", [C, TH], bf16, kind="ExternalInput")
    d_cond = nc.dram_tensor("cond", [C, T], bf16, kind="ExternalInput")
    d_w = {}
    for name in ("wql", "wkl", "wvl", "wqg", "wkg", "wvg"):
        d_w[name] = nc.dram_tensor(name, [C, C], bf16,
                                   kind="ExternalInput")
    d_bq = {}
    for name in ("bql", "bkl", "bqg", "bkg"):
        d_bq[name] = nc.dram_tensor(name, [C, 1], f32, kind="ExternalInput")
    d_bvl = nc.dram_tensor("bvl", [128, C], bf16, kind="ExternalInput")
    d_bvg = nc.dram_tensor("bvg", [128, C], bf16, kind="ExternalInput")
    d_wog = nc.dram_tensor("wog", [C, C], f32r, kind="ExternalInput")
    d_wol = nc.dram_tensor("wol", [C, C], bf16, kind="ExternalInput")
    d_bo = nc.dram_tensor("bo", [C, 1], f32, kind="ExternalInput")
    d_cosq = nc.dram_tensor("cosq", [128, TH], bf16, kind="ExternalInput")
    d_sinq = nc.dram_tensor("sinq", [128, TH], bf16, kind="ExternalInput")
    d_cosk = nc.dram_tensor("cosk", [128, T], bf16, kind="ExternalInput")
    d_sink = nc.dram_tensor("sink", [128, T], bf16, kind="ExternalInput")
    d_mask = nc.dram_tensor("mask", [128, 1280], bf16, kind="ExternalInput")
    d_ident = nc.dram_tensor("ident", [128, 64], f32r, kind="ExternalInput")
    d_out = nc.dram_tensor("out", [C, TH], f32, kind="ExternalOutput")

    SWAP_ADJ = [i ^ 1 for i in range(32)]
    NT_G = T // 128        # 16 global key tiles
    NT_L = KL // 128       # 17 local key tiles (last has 64 valid rows)

    with tile.TileContext(nc) as tc:
        with tc.tile_pool(name="persist", bufs=1) as pp, \
             tc.tile_pool(name="wpool", bufs=1) as wp, \
             tc.tile_pool(name="chunk", bufs=1) as cp, \
             tc.tile_pool(name="psA", bufs=2, space="PSUM") as psA, \
             tc.tile_pool(name="psB", bufs=2, space="PSUM") as psB, \
             tc.tile_pool(name="psC", bufs=2, space="PSUM") as psC:

            # ---------- persistent inputs ----------
            def load_rows(dram, rows, cols, dt):
                tiles = []
                nt = (rows + 127) // 128
                for i in range(nt):
                    p = min(128, rows - i * 128)
                    t_ = pp.tile([p, cols], dt, tag=f"ld_{dram.name}_{i}",
                                 name=f"ld_{dram.name}_{i}")
                    nc.sync.dma_start(t_[:], dram.ap()[i * 128:i * 128 + p, :])
                    tiles.append(t_)
                return tiles

            x_sb = load_rows(d_x, C, TH, bf16)          # 4x[128,TH]
            xq_sb = load_rows(d_xq, C, TH, bf16)
            cond_sb = load_rows(d_cond, C, T, bf16)     # 4x[128,T]
            cosq = load_rows(d_cosq, 128, TH, bf16)[0]
            sinq = load_rows(d_sinq, 128, TH, bf16)[0]
            cosk = load_rows(d_cosk, 128, T, bf16)[0]
            sink = load_rows(d_sink, 128, T, bf16)[0]
            mask_sb = load_rows(d_mask, 128, 1280, bf16)[0]
            bo_sb = load_rows(d_bo, C, 1, f32)          # 4x[128,1]
            ident = load_rows(d_ident, 128, 64, f32r)[0]
            ones_f32 = pp.tile([1, 64], f32, name="ones_f32")
            nc.vector.memset(ones_f32[:], 1.0)
            ones64 = pp.tile([1, 64], f32r, name="ones64")
            nc.vector.tensor_copy(ones64[:], ones_f32[:])
            b_sb = {nm: load_rows(d_bq[nm], C, 1, f32)
                    for nm in ("bql", "bkl", "bqg", "bkg")}
            bvl_sb = load_rows(d_bvl, 128, C, bf16)[0]
            bvg_sb = load_rows(d_bvg, 128, C, bf16)[0]

            onescol = pp.tile([128, 4], f32, name="onescol")
            nc.vector.memset(onescol[:], 1.0)
            zerot = pp.tile([128, 128], f32, name="zerot")
            nc.vector.memset(zerot[:], 0.0)

            # attention outputs, assembled in (C, TH) layout
            av_g = [pp.tile([128, TH], f32r, tag=f"avg{i}", name=f"avg{i}")
                    for i in range(4)]

            for hg in range(2):           # head group: 4 heads, 256 channels
                co = hg * 256

                # ---------- this head-group's weight slices ----------
                w_sb = {}
                for name in ("wql", "wkl", "wvl", "wqg", "wkg", "wvg"):
                    tiles = []
                    for i in range(4):
                        t_ = wp.tile([128, 256], bf16, tag=f"w{name}_{i}",
                                     name=f"w{name}_{i}")
                        nc.sync.dma_start(
                            t_[:],
                            d_w[name].ap()[i * 128:(i + 1) * 128, co:co + 256])
                        tiles.append(t_)
                    w_sb[name] = tiles

                # ---------- (C,T)-layout projections ----------
                def proj_ct(wname, bname, src_sb, tlen, dst_tiles,
                            dst_off=0, scale=None, rope=None):
                    # q (scale set): bias+scale fused into the ACT move.
                    # k: bias added by DVE scalar_tensor_tensor (bypass in1).
                    w = w_sb[wname]
                    for o in range(2):
                        bt = b_sb[bname][(co + o * 128) // 128]
                        for t0 in range(0, tlen, 512):
                            n = min(512, tlen - t0)
                            ps = psA.tile([128, n], f32, tag="proj",
                                          name="proj_ps")
                            for ct in range(4):
                                nc.tensor.matmul(
                                    ps[:], w[ct][:, o * 128:(o + 1) * 128],
                                    src_sb[ct][:, t0:t0 + n],
                                    start=(ct == 0), stop=(ct == 3))
                            dst = dst_tiles[o][:, dst_off + t0:
                                               dst_off + t0 + n]
                            if scale is not None:
                                nc.scalar.activation(dst, ps[:], IDENT,
                                                     bias=bt[:],
                                                     scale=scale)
                            else:
                                nc.vector.scalar_tensor_tensor(
                                    dst, ps[:], bt[:], cosk[:, 0:n],
                                    mybir.AluOpType.add,
                                    mybir.AluOpType.bypass)
                            if rope is not None:
                                ctab, stab = rope
                                shuf = cp.tile([128, n], f32r, tag="rshuf",
                                               name="rshuf", bufs=1)
                                nc.vector.stream_shuffle(
                                    shuf[:].bitcast(f32), dst.bitcast(f32),
                                    SWAP_ADJ)
                                nc.vector.tensor_mul(dst, dst,
                                                     ctab[:, t0:t0 + n])
                                nc.gpsimd.tensor_mul(shuf[:], shuf[:],
                                                     stab[:, t0:t0 + n])
                                nc.vector.tensor_add(dst, dst, shuf[:])

                q_l = [cp.tile([128, TH], f32r, tag=f"ql{o}", name=f"ql{o}")
                       for o in range(2)]
                av_l = [cp.tile([128, TH], f32r, tag=f"avl{i}",
                                name=f"avl{i}_{hg}") for i in range(2)]
                k_l = [cp.tile([128, KL], f32r, tag=f"kl{o}", name=f"kl{o}")
                       for o in range(2)]
                q_g = [cp.tile([128, TH], f32r, tag=f"qg{o}", name=f"qg{o}")
                       for o in range(2)]
                k_g = [cp.tile([128, T], f32r, tag=f"kg{o}", name=f"kg{o}")
                       for o in range(2)]

                # local K occupies columns [64, 2112) = t + 64; zero edges
                for o in range(2):
                    nc.vector.tensor_copy(k_l[o][:, 0:64], zerot[:, 0:64])
                    nc.vector.tensor_copy(k_l[o][:, T + 64:KL],
                                          zerot[:, 0:64])

                proj_ct("wql", "bql", xq_sb, TH, q_l, scale=SCALE)
                proj_ct("wkl", "bkl", cond_sb, T, k_l, dst_off=64)
                proj_ct("wqg", "bqg", x_sb, TH, q_g, scale=SCALE,
                        rope=(cosq, sinq))
                proj_ct("wkg", "bkg", cond_sb, T, k_g, rope=(cosk, sink))

                # ---------- transposed V projections (T, C+ones) ----------
                def proj_vt(wname, bv_sb, tlen, col_off):
                    # v tile tt rows p cover key-cols [128tt, 128tt+p);
                    # cond col = key-col - col_off.
                    w = w_sb[wname]
                    bvv = bv_sb[:, co:co + 256].rearrange(
                        "p (h d) -> p h d", d=64)
                    nt = (tlen + 127) // 128
                    tiles = []
                    for tt in range(nt):
                        p = min(128, tlen - tt * 128)
                        vt = cp.tile([128, 4 * 65], f32r,
                                     tag=f"vt_{wname}_{tt}",
                                     name=f"vt_{wname}_{tt}")
                        cstart = max(0, tt * 128 - col_off)
                        cend = min(T, tt * 128 + p - col_off)
                        ro = cstart - (tt * 128 - col_off)
                        m = cend - cstart
                        ps = psA.tile([128, 256], f32, tag="proj",
                                      name="proj_ps2")
                        for ct in range(4):
                            nc.tensor.matmul(
                                ps[ro:ro + m, :],
                                cond_sb[ct][:, cstart:cend],
                                w[ct][:, :], start=(ct == 0), stop=(ct == 3))
                        vda = vt[:, :].rearrange("p (h u) -> p h u", u=65)
                        nc.vector.tensor_add(
                            vda[ro:ro + m, :, 0:64],
                            ps[ro:ro + m, :].rearrange("p (h d) -> p h d",
                                                       d=64),
                            bvv[ro:ro + m])
                        # zero pad rows of the v columns
                        zview = zerot[:, 0:128].rearrange(
                            "p (h d) -> p h d", d=64)
                        if ro > 0:
                            nc.vector.tensor_copy(vda[0:ro, 0:2, 0:64],
                                                  zview[0:ro])
                            nc.vector.tensor_copy(vda[0:ro, 2:4, 0:64],
                                                  zview[0:ro])
                        if ro + m < p:
                            nc.vector.tensor_copy(vda[ro + m:p, 0:2, 0:64],
                                                  zview[ro + m:p])
                            nc.vector.tensor_copy(vda[ro + m:p, 2:4, 0:64],
                                                  zview[ro + m:p])
                        nc.vector.tensor_copy(
                            vda[0:p, :, 64:65],
                            onescol[0:p, :].rearrange("p (h u) -> p h u", u=1))
                        tiles.append(vt)
                    return tiles

                v_lT = proj_vt("wvl", bvl_sb, KL, 64)
                v_gT = proj_vt("wvg", bvg_sb, T, 0)

                # ---------- local attention (4 heads, W=16 band) ----------
                # query j (0..1023) = strided position t = 32*(j//16) + r0
                # + j%16; key tiles per 256-query block: 5 tiles of 128.
                for hp in range(2):        # head pair: rows 0-63 / 64-127
                    for qb in range(4):
                        j0 = qb * 256
                        ps_av = []
                        for ih in range(2):
                            hh = 2 * hp + ih
                            r0 = ih * 64
                            pe = cp.tile([128, 1280], f32r, tag="pe",
                                         name="lpe", bufs=2)
                            ps_s4 = psB.tile([128, 1024], f32, tag="gs",
                                             name="ls_ps4")
                            ps_s1 = psA.tile([128, 256], f32, tag="proj",
                                             name="ls_ps1")
                            for ti in range(5):
                                ks = qb * 512 + ti * 128
                                kw = min(128, KL - ks)
                                dst = (ps_s4[0:kw, ti * 256:(ti + 1) * 256]
                                       if ti < 4 else ps_s1[0:kw, :])
                                nc.tensor.matmul(
                                    dst,
                                    k_l[hp][r0:r0 + 64, ks:ks + kw],
                                    q_l[hp][r0:r0 + 64, j0:j0 + 256],
                                    start=True, stop=True)
                            nc.vector.tensor_add(pe[:, 0:1024], ps_s4[:],
                                                 mask_sb[:, 0:1024])
                            nc.vector.tensor_add(pe[:, 1024:1280], ps_s1[:],
                                                 mask_sb[:, 1024:1280])
                            nc.scalar.activation(pe[:], pe[:], EXP)
                            av = psC.tile([65, 512], f32, tag="av",
                                          name="lav_ps")
                            for ti in range(5):
                                ks = qb * 512 + ti * 128
                                kw = min(128, KL - ks)
                                nc.tensor.matmul(
                                    av[:, 0:256],
                                    v_lT[ks // 128][0:kw,
                                                    hh * 65:(hh + 1) * 65],
                                    pe[0:kw, ti * 256:(ti + 1) * 256],
                                    start=(ti == 0), stop=(ti == 4))
                            ps_av.append(av)
                        for ih in range(2):
                            rl = cp.tile([1, 512], f32r, tag="rec",
                                         name="lrec", bufs=1)
                            with nc.allow_low_precision(reason="f32r recip"):
                                nc.vector.reciprocal(rl[0:1, 0:256],
                                                     ps_av[ih][64:65, 0:256])
                            ps_bc = psA.tile([64, 512], f32, tag="proj",
                                             name="lbc_ps")
                            nc.tensor.matmul(ps_bc[:, 0:256], ones64[:],
                                             rl[0:1, 0:256],
                                             start=True, stop=True)
                            bc = cp.tile([64, 512], f32, tag="bcs",
                                         name="lbcs", bufs=1)
                            nc.scalar.copy(bc[:, 0:256], ps_bc[:, 0:256])
                            nc.vector.tensor_mul(
                                av_l[hp][ih * 64:ih * 64 + 64, j0:j0 + 256],
                                ps_av[ih][0:64, 0:256], bc[:, 0:256])

                # ---------- global attention (4 heads, full T keys) --------
                for hp in range(2):
                    ci = 2 * hg + hp
                    for qb in range(2):
                        j0 = qb * 512
                        ps_av = []
                        for ih in range(2):
                            hh = 2 * hp + ih
                            r0 = ih * 64
                            av = psC.tile([65, 512], f32, tag="av",
                                          name="gav_ps")
                            for kt2 in range(NT_G // 2):
                                ps_s = psB.tile([128, 1024], f32, tag="gs",
                                                name="gs_ps")
                                for u in range(2):
                                    kt = 2 * kt2 + u
                                    nc.tensor.matmul(
                                        ps_s[:, u * 512:(u + 1) * 512],
                                        k_g[hp][r0:r0 + 64,
                                                kt * 128:(kt + 1) * 128],
                                        q_g[hp][r0:r0 + 64, j0:j0 + 512],
                                        start=True, stop=True)
                                pe = cp.tile([128, 1024], f32r, tag="pe",
                                             name="gpe", bufs=2)
                                nc.scalar.activation(pe[:], ps_s[:], EXP)
                                for u in range(2):
                                    kt = 2 * kt2 + u
                                    nc.tensor.matmul(
                                        av[:],
                                        v_gT[kt][:, hh * 65:(hh + 1) * 65],
                                        pe[:, u * 512:(u + 1) * 512],
                                        start=(kt == 0),
                                        stop=(kt == NT_G - 1))
                            ps_av.append(av)
                        for ih in range(2):
                            rl = cp.tile([1, 512], f32r, tag="rec",
                                         name="grec", bufs=1)
                            with nc.allow_low_precision(reason="f32r recip"):
                                nc.vector.reciprocal(rl[:],
                                                     ps_av[ih][64:65, :])
                            ps_bc = psA.tile([64, 512], f32, tag="proj",
                                             name="gbc_ps")
                            nc.tensor.matmul(ps_bc[:], ones64[:], rl[:],
                                             start=True, stop=True)
                            bc = cp.tile([64, 512], f32, tag="bcs",
                                         name="gbcs", bufs=1)
                            nc.scalar.copy(bc[:], ps_bc[:])
                            nc.vector.tensor_mul(
                                av_g[ci][ih * 64:ih * 64 + 64, j0:j0 + 512],
                                ps_av[ih][0:64, :], bc[:])

                # ---------- local-branch channel scramble ----------
                # scr[64h + a, 64rr + d] = av_l[64h + d, 16a + rr]
                scr = [cp.tile([128, TH], bf16, tag=f"scr{o}",
                               name=f"scr{o}_{hg}", bufs=2)
                       for o in range(2)]
                for hh in range(4):
                    r0 = (hh % 2) * 64
                    src = av_l[hh // 2][r0:r0 + 64, :].rearrange(
                        "p (a rr) -> p rr a", rr=16)
                    for rr8 in range(2):
                        ps_t = psB.tile([64, 512], f32r, tag="gs",
                                        name="scr_ps")
                        for k in range(8):
                            rr = rr8 * 8 + k
                            nc.tensor.transpose(
                                ps_t[:, k * 64:(k + 1) * 64],
                                src[:, rr, :], ident[r0:r0 + 64, :])
                        nc.vector.tensor_copy(
                            scr[hh // 2][r0:r0 + 64,
                                         rr8 * 512:(rr8 + 1) * 512],
                            ps_t[:])

                # scr feeds the local output projection in place of av_l
                if hg == 0:
                    scr_all = [scr[0], scr[1], None, None]
                else:
                    scr_all = scr_all[:2] + [scr[0], scr[1]]

            # ---------- output projections (accumulate global + local) ----
            wog_sb = []
            wol_sb = []
            for i in range(4):
                t_ = wp.tile([128, 512], f32r, tag=f"wog{i}", name=f"wog{i}")
                nc.sync.dma_start(t_[:], d_wog.ap()[i * 128:(i + 1) * 128, :])
                wog_sb.append(t_)
                t_ = wp.tile([128, 512], bf16, tag=f"wol{i}", name=f"wol{i}")
                nc.sync.dma_start(t_[:], d_wol.ap()[i * 128:(i + 1) * 128, :])
                wol_sb.append(t_)

            for o in range(4):
                for tb in range(2):
                    t0 = tb * 512
                    ps = psA.tile([128, 512], f32, tag="proj", name="fin_ps")
                    for ct in range(4):
                        nc.tensor.matmul(
                            ps[:], wog_sb[ct][:, o * 128:(o + 1) * 128],
                            av_g[ct][:, t0:t0 + 512],
                            start=(ct == 0), stop=False)
                    for ct in range(4):
                        nc.tensor.matmul(
                            ps[:], wol_sb[ct][:, o * 128:(o + 1) * 128],
                            scr_all[ct][:, t0:t0 + 512],
                            start=False, stop=(ct == 3))
                    ot = cp.tile([128, 512], f32, tag="rshuf", name="outt",
                                 bufs=1)
                    nc.scalar.activation(ot[:], ps[:], IDENT,
                                         bias=bo_sb[o][:])
                    nc.sync.dma_start(d_out.ap()[o * 128:(o + 1) * 128,
                                                 t0:t0 + 512], ot[:])

    nc.compile()
    _prog_cache["nc"] = nc
    return nc


def _rope_tables():
    inv = 1.0 / (10000.0 ** (np.arange(0, D, 2, dtype=np.float64) / D))
    pos = np.arange(T, dtype=np.float64)
    ang = (pos[None, :] * inv[:, None]).astype(np.float32)  # (32, T)
    cosb = np.cos(ang).astype(np.float32)
    sinb = np.sin(ang).astype(np.float32)
    rows = (np.arange(128) % D) // 2
    c2 = cosb[rows]                       # (128, T)
    sign = np.where(np.arange(128) % 2 == 0, -1.0, 1.0).astype(np.float32)
    s2 = sinb[rows] * sign[:, None]
    return c2, s2


def _band_mask(r0):
    # S_T chunk ti (key cols 512qb+128ti + i) x query col j0+j:
    # t_q = 512qb + 32*(j//16) + r0 + j%16; key t = 512qb + 128ti + i - 64.
    # In-window iff key t - t_q in [-8, 7].
    m = np.full((128, 1280), NEG, dtype=np.float32)
    i = np.arange(128)[:, None]
    j = np.arange(256)[None, :]
    tq = 32 * (j // 16) + r0 + (j % 16)
    for ti in range(5):
        diff = (128 * ti + i - 64) - tq
        m[:, ti * 256:(ti + 1) * 256] = np.where(
            (diff >= -8) & (diff <= 7), 0.0, NEG)
    return m


def kernel(**inputs):
    import ml_dtypes
    bf = ml_dtypes.bfloat16

    x = np.ascontiguousarray(inputs["x"], dtype=np.float32)
    cond = np.ascontiguousarray(inputs["cond"], dtype=np.float32)
    t = np.asarray(inputs["t"]).astype(np.float64)

    def wT(w):
        return np.asarray(w, np.float32).T.astype(bf).copy()

    w_ql = wT(inputs["lq_w"])
    w_kl = wT(inputs["lk_w"])
    w_vl = wT(inputs["lv_w"])
    w_qg = wT(inputs["gq_w"])
    w_kg = wT(inputs["gk_w"])
    w_vg = wT(inputs["gv_w"])

    def col(b):
        return np.asarray(b, np.float32).reshape(C, 1).copy()

    b_ql = col(inputs["lq_b"]) * np.float32(SCALE)
    b_qg = col(inputs["gq_b"]) * np.float32(SCALE)
    b_kl = col(inputs["lk_b"])
    b_kg = col(inputs["gk_b"])
    bv_l = np.broadcast_to(np.asarray(inputs["lv_b"], np.float32),
                           (128, C)).astype(bf).copy()
    bv_g = np.broadcast_to(np.asarray(inputs["gv_b"], np.float32),
                           (128, C)).astype(bf).copy()


    t_norm = t / np.float64(TS - 1)
    sg = np.sqrt(1.0 - t_norm).astype(np.float32)   # (B,)
    sl = np.sqrt(t_norm).astype(np.float32)

    c2, s2 = _rope_tables()
    masks = [_band_mask(0).astype(bf), _band_mask(16).astype(bf)]
    ident = np.vstack([np.eye(64), np.eye(64)]).astype(np.float32)

    go_w = np.asarray(inputs["go_w"], np.float32)
    lo_w = np.asarray(inputs["lo_w"], np.float32)
    go_b = np.asarray(inputs["go_b"], np.float32)
    lo_b = np.asarray(inputs["lo_b"], np.float32)

    x_bf = x.astype(bf)
    cond_bf = cond.astype(bf)

    # strided local query columns: half h takes t with t%32 in [16h, 16h+16)
    tcols = np.arange(T).reshape(T // 32, 32)
    qsel = [tcols[:, 0:16].ravel(), tcols[:, 16:32].ravel()]

    in_maps = []
    for core in range(N_CORES):
        b = core // 2
        half = core % 2
        hs = half * TH
        in_maps.append({
            "x": np.ascontiguousarray(x_bf[b][:, hs:hs + TH]),
            "xq": np.ascontiguousarray(x_bf[b][:, qsel[half]]),
            "cond": cond_bf[b],
            "wql": w_ql, "wkl": w_kl, "wvl": w_vl,
            "wqg": w_qg, "wkg": w_kg, "wvg": w_vg,
            "bql": b_ql, "bkl": b_kl, "bqg": b_qg, "bkg": b_kg,
            "bvl": bv_l, "bvg": bv_g,
            "wog": (go_w.T * sg[b]).copy(),
            "wol": (lo_w.T * sl[b]).astype(bf),
            "bo": (sg[b] * go_b + sl[b] * lo_b).reshape(C, 1).copy(),
            "cosq": np.ascontiguousarray(c2[:, hs:hs + TH]).astype(bf),
            "sinq": np.ascontiguousarray(s2[:, hs:hs + TH]).astype(bf),
            "cosk": c2.astype(bf),
            "sink": s2.astype(bf),
            "mask": masks[half],
            "ident": ident,
        })

    nc = _build_program()
    from concourse.bass_utils import run_bass_kernel_spmd
    res = run_bass_kernel_spmd(nc, in_maps, list(range(N_CORES)))

    out = np.empty((B, C, T), np.float32)
    for core in range(N_CORES):
        b = core // 2
        hs = (core % 2) * TH
        out[b][:, hs:hs + TH] = res.results[core]["out"]
    return out


# revision 25
# speedup vs baseline: 10.3142x; 9.9906x over previous
"""Trainium2 Bass kernel for SoftAlignAttentionMixed.

Shapes: x, cond (4, 512, 2048); 8 projection weights (512, 512) + biases.
Computation = local windowed attention (W=16) + global RoPE attention,
mixed per-batch by sqrt(1-t/999) / sqrt(t/999).

Sharding: 8 cores = (batch b, T-half) grid. Each core computes, for its
batch, the global attention for 1024 contiguous query positions and the
local attention for the 1024 strided query positions {t : t%32 in
[16*half, 16*half+16)}. The strided split matches the reference's
torch-faithful channel scramble of the local branch: its
(B,H,T,D).view(B,C,T) maps head h, query t=32a+r, dim d to channel
64h+a, position 64r+d — so output columns [hs, hs+1024) need exactly the
local-attention results for queries with t%32 in the core's residue set.
No collectives; host gathers (C, 1024) output slabs.

On-device layout notes:
 - Projections keep channels-first (C, T): matmul lhsT = W^T tile, rhs =
   input, inputs/weights in bf16, accumulate f32, outputs stored f32r.
 - Biases applied by the PE via a bias row appended to W^T times a ones row.
 - Local K/V span padded columns [t=-64, t=2112); edge columns are zeroed on
   device, reproducing the reference's zero-padding softmax semantics
   (pad logits 0, pad v rows 0, denominators include pads).
 - V is projected directly into transposed (T, C) layout (lhsT = cond tile,
   rhs = W^T), with an extra ones-column per head so the attention AV
   matmul also produces softmax denominators (row 64 of its PSUM out).
 - Scores are computed transposed: S_T[tk, tq] = k_tile^T @ q (f32r), exp on
   ACT, AV contracts over tk partitions; normalization = reciprocal of the
   denominator row, rank-1 PE broadcast, one DVE multiply.
 - RoPE: interleaved pairs are adjacent partitions; pair swap via DVE
   stream_shuffle; out = x*C2 + swap(x)*S2 with host tables.
 - The local-branch scramble is materialized per head with 16 PE
   transposes of 64x64 column-strided blocks of the av map.
 - t-mixing folded into host-scaled output weights; both output projections
   accumulate into one PSUM tile.
"""

import numpy as np

B, C, T, H, W, TS = 4, 512, 2048, 8, 16, 1000
D = C // H            # 64
TH = T // 2           # 1024 queries per core (per branch)
KL = T + 128          # 2176 local key cols (t = col-64, 64-col pads)
SCALE = D ** -0.5
N_CORES = 8
NEG = -1.0e30

_prog_cache = {}


def _build_program():
    if "nc" in _prog_cache:
        return _prog_cache["nc"]

    import concourse.bacc as bacc
    import concourse.mybir as mybir
    import concourse.tile as tile

    f32 = mybir.dt.float32
    f32r = mybir.dt.float32r
    bf16 = mybir.dt.bfloat16
    EXP = mybir.ActivationFunctionType.Exp
    IDENT = mybir.ActivationFunctionType.Identity

    nc = bacc.Bacc("TRN2", target_bir_lowering=False, debug=False,
                   num_devices=N_CORES)

    # ---- DRAM I/O (per-core tensors; same program on all cores) ----
    d_x = nc.dram_tensor("x", [C, TH], bf16, kind="ExternalInput")
    d_xq = nc.dram_tensor("xq", [C, TH], bf16, kind="ExternalInput")
    d_cond = nc.dram_tensor("cond", [C, T], bf16, kind="ExternalInput")
    d_w = {}
    for name in ("wql", "wkl", "wvl", "wqg", "wkg", "wvg"):
        d_w[name] = nc.dram_tensor(name, [C, C], bf16,
                                   kind="ExternalInput")
    d_bq = {}
    for name in ("bql", "bkl", "bqg", "bkg"):
        d_bq[name] = nc.dram_tensor(name, [C, 1], f32, kind="ExternalInput")
    d_bvl = nc.dram_tensor("bvl", [128, C], bf16, kind="ExternalInput")
    d_bvg = nc.dram_tensor("bvg", [128, C], bf16, kind="ExternalInput")
    d_wog = nc.dram_tensor("wog", [C, C], f32r, kind="ExternalInput")
    d_wol = nc.dram_tensor("wol", [C, C], bf16, kind="ExternalInput")
    d_bo = nc.dram_tensor("bo", [C, 1], f32, kind="ExternalInput")
    d_cosq = nc.dram_tensor("cosq", [128, TH], bf16, kind="ExternalInput")
    d_sinq = nc.dram_tensor("sinq", [128, TH], bf16, kind="ExternalInput")
    d_cosk = nc.dram_tensor("cosk", [128, T], bf16, kind="ExternalInput")
    d_sink = nc.dram_tensor("sink", [128, T], bf16, kind="ExternalInput")
    d_mask = nc.dram_tensor("mask", [128, 1280], bf16, kind="ExternalInput")
    d_ident = nc.dram_tensor("ident", [128, 64], f32r, kind="ExternalInput")
    d_out = nc.dram_tensor("out", [C, TH], f32, kind="ExternalOutput")

    SWAP_ADJ = [i ^ 1 for i in range(32)]
    NT_G = T // 128        # 16 global key tiles
    NT_L = KL // 128       # 17 local key tiles (last has 64 valid rows)

    with tile.TileContext(nc) as tc:
        with tc.tile_pool(name="persist", bufs=1) as pp, \
             tc.tile_pool(name="wpool", bufs=1) as wp, \
             tc.tile_pool(name="chunk", bufs=1) as cp, \
             tc.tile_pool(name="psA", bufs=2, space="PSUM") as psA, \
             tc.tile_pool(name="psB", bufs=2, space="PSUM") as psB, \
             tc.tile_pool(name="psC", bufs=2, space="PSUM") as psC:

            # ---------- persistent inputs ----------
            def load_rows(dram, rows, cols, dt):
                tiles = []
                nt = (rows + 127) // 128
                for i in range(nt):
                    p = min(128, rows - i * 128)
                    t_ = pp.tile([p, cols], dt, tag=f"ld_{dram.name}_{i}",
                                 name=f"ld_{dram.name}_{i}")
                    nc.sync.dma_start(t_[:], dram.ap()[i * 128:i * 128 + p, :])
                    tiles.append(t_)
                return tiles

            x_sb = load_rows(d_x, C, TH, bf16)          # 4x[128,TH]
            xq_sb = load_rows(d_xq, C, TH, bf16)
            cond_sb = load_rows(d_cond, C, T, bf16)     # 4x[128,T]
            cosq = load_rows(d_cosq, 128, TH, bf16)[0]
            sinq = load_rows(d_sinq, 128, TH, bf16)[0]
            cosk = load_rows(d_cosk, 128, T, bf16)[0]
            sink = load_rows(d_sink, 128, T, bf16)[0]
            mask_sb = load_rows(d_mask, 128, 1280, bf16)[0]
            bo_sb = load_rows(d_bo, C, 1, f32)          # 4x[128,1]
            ident = load_rows(d_ident, 128, 64, f32r)[0]
            ones_f32 = pp.tile([1, 64], f32, name="ones_f32")
            nc.vector.memset(ones_f32[:], 1.0)
            ones64 = pp.tile([1, 64], f32r, name="ones64")
            nc.vector.tensor_copy(ones64[:], ones_f32[:])
            b_sb = {nm: load_rows(d_bq[nm], C, 1, f32)
                    for nm in ("bql", "bkl", "bqg", "bkg")}
            bvl_sb = load_rows(d_bvl, 128, C, bf16)[0]
            bvg_sb = load_rows(d_bvg, 128, C, bf16)[0]

            onescol = pp.tile([128, 4], f32, name="onescol")
            nc.vector.memset(onescol[:], 1.0)
            zerot = pp.tile([128, 128], f32, name="zerot")
            nc.vector.memset(zerot[:], 0.0)

            # attention outputs, assembled in (C, TH) layout
            av_g = [pp.tile([128, TH], f32r, tag=f"avg{i}", name=f"avg{i}")
                    for i in range(4)]

            for hg in range(2):           # head group: 4 heads, 256 channels
                co = hg * 256

                # ---------- this head-group's weight slices ----------
                w_sb = {}
                for name in ("wql", "wkl", "wvl", "wqg", "wkg", "wvg"):
                    tiles = []
                    for i in range(4):
                        t_ = wp.tile([128, 256], bf16, tag=f"w{name}_{i}",
                                     name=f"w{name}_{i}")
                        nc.sync.dma_start(
                            t_[:],
                            d_w[name].ap()[i * 128:(i + 1) * 128, co:co + 256])
                        tiles.append(t_)
                    w_sb[name] = tiles

                # ---------- (C,T)-layout projections ----------
                def proj_ct(wname, bname, src_sb, tlen, dst_tiles,
                            dst_off=0, scale=None, rope=None):
                    # q (scale set): bias+scale fused into the ACT move.
                    # k: bias added by DVE scalar_tensor_tensor (bypass in1).
                    w = w_sb[wname]
                    for o in range(2):
                        bt = b_sb[bname][(co + o * 128) // 128]
                        for t0 in range(0, tlen, 512):
                            n = min(512, tlen - t0)
                            ps = psA.tile([128, n], f32, tag="proj",
                                          name="proj_ps")
                            for ct in range(4):
                                nc.tensor.matmul(
                                    ps[:], w[ct][:, o * 128:(o + 1) * 128],
                                    src_sb[ct][:, t0:t0 + n],
                                    start=(ct == 0), stop=(ct == 3))
                            dst = dst_tiles[o][:, dst_off + t0:
                                               dst_off + t0 + n]
                            if scale is not None:
                                nc.scalar.activation(dst, ps[:], IDENT,
                                                     bias=bt[:],
                                                     scale=scale)
                            else:
                                nc.vector.scalar_tensor_tensor(
                                    dst, ps[:], bt[:], cosk[:, 0:n],
                                    mybir.AluOpType.add,
                                    mybir.AluOpType.bypass)
                            if rope is not None:
                                ctab, stab = rope
                                shuf = cp.tile([128, n], f32r, tag="rshuf",
                                               name="rshuf", bufs=1)
                                nc.vector.stream_shuffle(
                                    shuf[:].bitcast(f32), dst.bitcast(f32),
                                    SWAP_ADJ)
                                nc.vector.tensor_mul(dst, dst,
                                                     ctab[:, t0:t0 + n])
                                nc.gpsimd.tensor_mul(shuf[:], shuf[:],
                                                     stab[:, t0:t0 + n])
                                nc.vector.tensor_add(dst, dst, shuf[:])

                q_l = [cp.tile([128, TH], f32r, tag=f"ql{o}", name=f"ql{o}")
                       for o in range(2)]
                av_l = [cp.tile([128, TH], f32r, tag=f"avl{i}",
                                name=f"avl{i}_{hg}") for i in range(2)]
                k_l = [cp.tile([128, KL], f32r, tag=f"kl{o}", name=f"kl{o}")
                       for o in range(2)]
                q_g = [cp.tile([128, TH], f32r, tag=f"qg{o}", name=f"qg{o}")
                       for o in range(2)]
                k_g = [cp.tile([128, T], f32r, tag=f"kg{o}", name=f"kg{o}")
                       for o in range(2)]

                # local K occupies columns [64, 2112) = t + 64; zero edges
                for o in range(2):
                    nc.vector.tensor_copy(k_l[o][:, 0:64], zerot[:, 0:64])
                    nc.vector.tensor_copy(k_l[o][:, T + 64:KL],
                                          zerot[:, 0:64])

                proj_ct("wql", "bql", xq_sb, TH, q_l, scale=SCALE)
                proj_ct("wkl", "bkl", cond_sb, T, k_l, dst_off=64)
                proj_ct("wqg", "bqg", x_sb, TH, q_g, scale=SCALE,
                        rope=(cosq, sinq))
                proj_ct("wkg", "bkg", cond_sb, T, k_g, rope=(cosk, sink))

                # ---------- transposed V projections (T, C+ones) ----------
                def proj_vt(wname, bv_sb, tlen, col_off):
                    # v tile tt rows p cover key-cols [128tt, 128tt+p);
                    # cond col = key-col - col_off.
                    w = w_sb[wname]
                    bvv = bv_sb[:, co:co + 256].rearrange(
                        "p (h d) -> p h d", d=64)
                    nt = (tlen + 127) // 128
                    tiles = []
                    for tt in range(nt):
                        p = min(128, tlen - tt * 128)
                        vt = cp.tile([128, 4 * 65], f32r,
                                     tag=f"vt_{wname}_{tt}",
                                     name=f"vt_{wname}_{tt}")
                        cstart = max(0, tt * 128 - col_off)
                        cend = min(T, tt * 128 + p - col_off)
                        ro = cstart - (tt * 128 - col_off)
                        m = cend - cstart
                        ps = psA.tile([128, 256], f32, tag="proj",
                                      name="proj_ps2")
                        for ct in range(4):
                            nc.tensor.matmul(
                                ps[ro:ro + m, :],
                                cond_sb[ct][:, cstart:cend],
                                w[ct][:, :], start=(ct == 0), stop=(ct == 3))
                        vda = vt[:, :].rearrange("p (h u) -> p h u", u=65)
                        nc.vector.tensor_add(
                            vda[ro:ro + m, :, 0:64],
                            ps[ro:ro + m, :].rearrange("p (h d) -> p h d",
                                                       d=64),
                            bvv[ro:ro + m])
                        # zero pad rows of the v columns
                        zview = zerot[:, 0:128].rearrange(
                            "p (h d) -> p h d", d=64)
                        if ro > 0:
                            nc.vector.tensor_copy(vda[0:ro, 0:2, 0:64],
                                                  zview[0:ro])
                            nc.vector.tensor_copy(vda[0:ro, 2:4, 0:64],
                                                  zview[0:ro])
                        if ro + m < p:
                            nc.vector.tensor_copy(vda[ro + m:p, 0:2, 0:64],
                                                  zview[ro + m:p])
                            nc.vector.tensor_copy(vda[ro + m:p, 2:4, 0:64],
                                                  zview[ro + m:p])
                        nc.vector.tensor_copy(
                            vda[0:p, :, 64:65],
                            onescol[0:p, :].rearrange("p (h u) -> p h u", u=1))
                        tiles.append(vt)
                    return tiles

                v_lT = proj_vt("wvl", bvl_sb, KL, 64)
                v_gT = proj_vt("wvg", bvg_sb, T, 0)

                # ---------- local attention (4 heads, W=16 band) ----------
                # query j (0..1023) = strided position t = 32*(j//16) + r0
                # + j%16; key tiles per 256-query block: 5 tiles of 128.
                for hp in range(2):        # head pair: rows 0-63 / 64-127
                    for qb in range(4):
                        j0 = qb * 256
                        ps_av = []
                        for ih in range(2):
                            hh = 2 * hp + ih
                            r0 = ih * 64
                            pe = cp.tile([128, 1280], f32r, tag="pe",
                                         name="lpe", bufs=2)
                            ps_s4 = psB.tile([128, 1024], f32, tag="gs",
                                             name="ls_ps4")
                            ps_s1 = psA.tile([128, 256], f32, tag="proj",
                                             name="ls_ps1")
                            for ti in range(5):
                                ks = qb * 512 + ti * 128
                                kw = min(128, KL - ks)
                                dst = (ps_s4[0:kw, ti * 256:(ti + 1) * 256]
                                       if ti < 4 else ps_s1[0:kw, :])
                                nc.tensor.matmul(
                                    dst,
                                    k_l[hp][r0:r0 + 64, ks:ks + kw],
                                    q_l[hp][r0:r0 + 64, j0:j0 + 256],
                                    start=True, stop=True)
                            nc.vector.tensor_add(pe[:, 0:1024], ps_s4[:],
                                                 mask_sb[:, 0:1024])
                            nc.vector.tensor_add(pe[:, 1024:1280], ps_s1[:],
                                                 mask_sb[:, 1024:1280])
                            nc.scalar.activation(pe[:], pe[:], EXP)
                            av = psC.tile([65, 512], f32, tag="av",
                                          name="lav_ps")
                            for ti in range(5):
                                ks = qb * 512 + ti * 128
                                kw = min(128, KL - ks)
                                nc.tensor.matmul(
                                    av[:, 0:256],
                                    v_lT[ks // 128][0:kw,
                                                    hh * 65:(hh + 1) * 65],
                                    pe[0:kw, ti * 256:(ti + 1) * 256],
                                    start=(ti == 0), stop=(ti == 4))
                            ps_av.append(av)
                        for ih in range(2):
                            rl = cp.tile([1, 512], f32r, tag="rec",
                                         name="lrec", bufs=1)
                            with nc.allow_low_precision(reason="f32r recip"):
                                nc.vector.reciprocal(rl[0:1, 0:256],
                                                     ps_av[ih][64:65, 0:256])
                            ps_bc = psA.tile([64, 512], f32, tag="proj",
                                             name="lbc_ps")
                            nc.tensor.matmul(ps_bc[:, 0:256], ones64[:],
                                             rl[0:1, 0:256],
                                             start=True, stop=True)
                            bc = cp.tile([64, 512], f32, tag="bcs",
                                         name="lbcs", bufs=1)
                            nc.scalar.copy(bc[:, 0:256], ps_bc[:, 0:256])
                            nc.vector.tensor_mul(
                                av_l[hp][ih * 64:ih * 64 + 64, j0:j0 + 256],
                                ps_av[ih][0:64, 0:256], bc[:, 0:256])

                # ---------- global attention (4 heads, full T keys) --------
                for hp in range(2):
                    ci = 2 * hg + hp
                    for qb in range(2):
                        j0 = qb * 512
                        ps_av = []
                        for ih in range(2):
                            hh = 2 * hp + ih
                            r0 = ih * 64
                            av = psC.tile([65, 512], f32, tag="av",
                                          name="gav_ps")
                            for kt2 in range(NT_G // 2):
                                ps_s = psB.tile([128, 1024], f32, tag="gs",
                                                name="gs_ps")
                                for u in range(2):
                                    kt = 2 * kt2 + u
                                    nc.tensor.matmul(
                                        ps_s[:, u * 512:(u + 1) * 512],
                                        k_g[hp][r0:r0 + 64,
                                                kt * 128:(kt + 1) * 128],
                                        q_g[hp][r0:r0 + 64, j0:j0 + 512],
                                        start=True, stop=True)
                                pe = cp.tile([128, 1024], f32r, tag="pe",
                                             name="gpe", bufs=2)
                                nc.scalar.activation(pe[:], ps_s[:], EXP)
                                for u in range(2):
                                    kt = 2 * kt2 + u
                                    nc.tensor.matmul(
                                        av[:],
                                        v_gT[kt][:, hh * 65:(hh + 1) * 65],
                                        pe[:, u * 512:(u + 1) * 512],
                                        start=(kt == 0),
                                        stop=(kt == NT_G - 1))
                            ps_av.append(av)
                        for ih in range(2):
                            rl = cp.tile([1, 512], f32r, tag="rec",
                                         name="grec", bufs=1)
                            with nc.allow_low_precision(reason="f32r recip"):
                                nc.vector.reciprocal(rl[:],
                                                     ps_av[ih][64:65, :])
                            ps_bc = psA.tile([64, 512], f32, tag="proj",
                                             name="gbc_ps")
                            nc.tensor.matmul(ps_bc[:], ones64[:], rl[:],
                                             start=True, stop=True)
                            bc = cp.tile([64, 512], f32, tag="bcs",
                                         name="gbcs", bufs=1)
                            nc.scalar.copy(bc[:], ps_bc[:])
                            nc.vector.tensor_mul(
                                av_g[ci][ih * 64:ih * 64 + 64, j0:j0 + 512],
                                ps_av[ih][0:64, :], bc[:])

                # ---------- local-branch channel scramble ----------
                # scr[64h + a, 64rr + d] = av_l[64h + d, 16a + rr]
                scr = [cp.tile([128, TH], bf16, tag=f"scr{o}",
                               name=f"scr{o}_{hg}", bufs=2)
                       for o in range(2)]
                for hh in range(4):
                    r0 = (hh % 2) * 64
                    src = av_l[hh // 2][r0:r0 + 64, :].rearrange(
                        "p (a rr) -> p rr a", rr=16)
                    for rr8 in range(2):
                        ps_t = psB.tile([64, 512], f32r, tag="gs",
                                        name="scr_ps")
                        for k in range(8):
                            rr = rr8 * 8 + k
                            nc.tensor.transpose(
                                ps_t[:, k * 64:(k + 1) * 64],
                                src[:, rr, :], ident[r0:r0 + 64, :])
                        nc.vector.tensor_copy(
                            scr[hh // 2][r0:r0 + 64,
                                         rr8 * 512:(rr8 + 1) * 512],
                            ps_t[:])

                # scr feeds the local output projection in place of av_l
                if hg == 0:
                    scr_all = [scr[0], scr[1], None, None]
                else:
                    scr_all = scr_all[:2] + [scr[0], scr[1]]

            # ---------- output projections (accumulate global + local) ----
            wog_sb = []
            wol_sb = []
            for i in range(4):
                t_ = wp.tile([128, 512], f32r, tag=f"wog{i}", name=f"wog{i}")
                nc.sync.dma_start(t_[:], d_wog.ap()[i * 128:(i + 1) * 128, :])
                wog_sb.append(t_)
                t_ = wp.tile([128, 512], bf16, tag=f"wol{i}", name=f"wol{i}")
                nc.sync.dma_start(t_[:], d_wol.ap()[i * 128:(i + 1) * 128, :])
                wol_sb.append(t_)

            for o in range(4):
                for tb in range(2):
                    t0 = tb * 512
                    ps = psA.tile([128, 512], f32, tag="proj", name="fin_ps")
                    for ct in range(4):
                        nc.tensor.matmul(
                            ps[:], wog_sb[ct][:, o * 128:(o + 1) * 128],
                            av_g[ct][:, t0:t0 + 512],
                            start=(ct == 0), stop=False)
                    for ct in range(4):
                        nc.tensor.matmul(
                            ps[:], wol_sb[ct][:, o * 128:(o + 1) * 128],
                            scr_all[ct][:, t0:t0 + 512],
                            start=False, stop=(ct == 3))
                    ot = cp.tile([128, 512], f32, tag="rshuf", name="outt",
                                 bufs=1)
                    nc.scalar.activation(ot[:], ps[:], IDENT,
                                         bias=bo_sb[o][:])
                    nc.sync.dma_start(d_out.ap()[o * 128:(o + 1) * 128,
                                                 t0:t0 + 512], ot[:])

    nc.compile()
    _prog_cache["nc"] = nc
    return nc


_table_cache = {}


def _rope_tables():
    if "t" in _table_cache:
        return _table_cache["t"]
    inv = 1.0 / (10000.0 ** (np.arange(0, D, 2, dtype=np.float64) / D))
    pos = np.arange(T, dtype=np.float64)
    ang = (pos[None, :] * inv[:, None]).astype(np.float32)  # (32, T)
    cosb = np.cos(ang).astype(np.float32)
    sinb = np.sin(ang).astype(np.float32)
    rows = (np.arange(128) % D) // 2
    c2 = cosb[rows]                       # (128, T)
    sign = np.where(np.arange(128) % 2 == 0, -1.0, 1.0).astype(np.float32)
    s2 = sinb[rows] * sign[:, None]
    _table_cache["t"] = (c2, s2)
    return c2, s2


def _band_mask(r0):
    # S_T chunk ti (key cols 512qb+128ti + i) x query col j0+j:
    # t_q = 512qb + 32*(j//16) + r0 + j%16; key t = 512qb + 128ti + i - 64.
    # In-window iff key t - t_q in [-8, 7].
    m = np.full((128, 1280), NEG, dtype=np.float32)
    i = np.arange(128)[:, None]
    j = np.arange(256)[None, :]
    tq = 32 * (j // 16) + r0 + (j % 16)
    for ti in range(5):
        diff = (128 * ti + i - 64) - tq
        m[:, ti * 256:(ti + 1) * 256] = np.where(
            (diff >= -8) & (diff <= 7), 0.0, NEG)
    return m


def kernel(**inputs):
    import ml_dtypes
    bf = ml_dtypes.bfloat16

    x = np.ascontiguousarray(inputs["x"], dtype=np.float32)
    cond = np.ascontiguousarray(inputs["cond"], dtype=np.float32)
    t = np.asarray(inputs["t"]).astype(np.float64)

    def wT(w):
        return np.asarray(w, np.float32).T.astype(bf).copy()

    w_ql = wT(inputs["lq_w"])
    w_kl = wT(inputs["lk_w"])
    w_vl = wT(inputs["lv_w"])
    w_qg = wT(inputs["gq_w"])
    w_kg = wT(inputs["gk_w"])
    w_vg = wT(inputs["gv_w"])

    def col(b):
        return np.asarray(b, np.float32).reshape(C, 1).copy()

    b_ql = col(inputs["lq_b"]) * np.float32(SCALE)
    b_qg = col(inputs["gq_b"]) * np.float32(SCALE)
    b_kl = col(inputs["lk_b"])
    b_kg = col(inputs["gk_b"])
    bv_l = np.broadcast_to(np.asarray(inputs["lv_b"], np.float32),
                           (128, C)).astype(bf).copy()
    bv_g = np.broadcast_to(np.asarray(inputs["gv_b"], np.float32),
                           (128, C)).astype(bf).copy()


    t_norm = t / np.float64(TS - 1)
    sg = np.sqrt(1.0 - t_norm).astype(np.float32)   # (B,)
    sl = np.sqrt(t_norm).astype(np.float32)

    c2, s2 = _rope_tables()
    masks = [_band_mask(0).astype(bf), _band_mask(16).astype(bf)]
    ident = np.vstack([np.eye(64), np.eye(64)]).astype(np.float32)

    go_w = np.asarray(inputs["go_w"], np.float32)
    lo_w = np.asarray(inputs["lo_w"], np.float32)
    go_b = np.asarray(inputs["go_b"], np.float32)
    lo_b = np.asarray(inputs["lo_b"], np.float32)

    x_bf = x.astype(bf)
    cond_bf = cond.astype(bf)

    # strided local query columns: half h takes t with t%32 in [16h, 16h+16)
    tcols = np.arange(T).reshape(T // 32, 32)
    qsel = [tcols[:, 0:16].ravel(), tcols[:, 16:32].ravel()]

    in_maps = []
    for core in range(N_CORES):
        b = core // 2
        half = core % 2
        hs = half * TH
        in_maps.append({
            "x": np.ascontiguousarray(x_bf[b][:, hs:hs + TH]),
            "xq": np.ascontiguousarray(x_bf[b][:, qsel[half]]),
            "cond": cond_bf[b],
            "wql": w_ql, "wkl": w_kl, "wvl": w_vl,
            "wqg": w_qg, "wkg": w_kg, "wvg": w_vg,
            "bql": b_ql, "bkl": b_kl, "bqg": b_qg, "bkg": b_kg,
            "bvl": bv_l, "bvg": bv_g,
            "wog": (go_w.T * sg[b]).copy(),
            "wol": (lo_w.T * sl[b]).astype(bf),
            "bo": (sg[b] * go_b + sl[b] * lo_b).reshape(C, 1).copy(),
            "cosq": np.ascontiguousarray(c2[:, hs:hs + TH]).astype(bf),
            "sinq": np.ascontiguousarray(s2[:, hs:hs + TH]).astype(bf),
            "cosk": c2.astype(bf),
            "sink": s2.astype(bf),
            "mask": masks[half],
            "ident": ident,
        })

    nc = _build_program()
    from concourse.bass_utils import run_bass_kernel_spmd
    res = run_bass_kernel_spmd(nc, in_maps, list(range(N_CORES)))

    out = np.empty((B, C, T), np.float32)
    for core in range(N_CORES):
        b = core // 2
        hs = (core % 2) * TH
        out[b][:, hs:hs + TH] = res.results[core]["out"]
    return out
